# revision 44
# baseline (speedup 1.0000x reference)
"""AttentionBlock (GroupNorm + single-head self-attention + residual) on 8 trn2 cores.

Data-parallel over batch: B=16 images, 2 per core; no collectives. All large
matmuls run as fp32r (FP22-truncated fp32), which streams at 1 col/cycle on
the PE for free dims >= 256 -- full bf16-rate with ~13-bit mantissa accuracy
(measured end-to-end rel err ~2.5e-5 vs the fp32 reference).

The four 512x512 projections are algebraically merged HOST-SIDE into two:
  logits  = scale * q^T k = hn^T (scale * wq^T wk) hn   -> one u-projection
  output  = wo @ (attn @ v) = attn @ ((wo wv) @ hn)     -> one v'-projection
so the device runs only 2 projection passes (u, v'T), no separate k/v
projections and NO output-projection phase at all. A nonzero bq adds a rank-1
per-column logit term; it is handled exactly (when present) as a per-m-tile
exp() bias computed by tiny N=1 matmuls of hn against scale * wk^T bq. bk is
dropped (constant per softmax row); bv folds into bo' = bo + wo@bv.

Per-image layouts (SBUF, partition x free):
  x, hn, u : [c, n] as 4 tiles [128, 1024]
  v'T      : [m, c'] as 8 tiles [128, 512] (hn-stationary matmuls, transposed
             for free)
  attnT    : exp(L^T)[m, n] as 8 tiles [128, 1024]

No PE transposes anywhere: logits are computed transposed (L^T = hn^T u per
128-row m-tile) so the ACT engine's Exp writes attnT straight from PSUM.
Softmax runs without max-subtraction (logits ~N(0,1) by construction).
Denominators: column sums of exp via ones-vector matmuls, broadcast to all
128 partitions with a K=1 outer-product matmul + fast approximate reciprocal;
the 1/sum multiply and the bias+residual add (x read from its GN-phase tiles) form the A@V'
PSUM->SBUF epilogue, streaming results out per [128, 512] chunk.

GroupNorm: per-channel sum/sumsq (DVE reduce + Square-accumulate split across
engines), group reduction and per-channel broadcast via tiny group-membership
matmuls. Both images' stats phases are emitted up front (tiny tiles double-
buffered); x loads are split across two DMA queues; a short burst of junk
fp32 matmuls warms the PE clock (HAM) during the initial DMA wait.
"""

import sys

sys.path.insert(0, "/opt/trn_rl_repo")

from contextlib import ExitStack

import numpy as np

import concourse.bass as bass
import concourse.bacc as bacc
import concourse.mybir as mybir
import concourse.tile as tile
from concourse.bass_utils import run_bass_kernel_spmd

B, C, H, W = 16, 512, 32, 32
HW = H * W  # 1024 pixels (n/m index)
NCORES = 8
BLOC = B // NCORES  # 2 images per core
G = 8  # groupnorm groups
GSZ = C // G  # 64 channels per group
SCALE = float(C) ** -0.5
EPS = 1e-5
INVCNT = 1.0 / (GSZ * HW)

F32 = mybir.dt.float32
F32R = mybir.dt.float32r
AF = mybir.ActivationFunctionType
ALU = mybir.AluOpType
AX = mybir.AxisListType

CT = C // 128  # 4 channel tiles
NB = HW // 128  # 8 row blocks of the attention matrix
NCH = HW // 512  # 2 free-dim chunks of 512


def r(ap):
    return ap.bitcast(F32R)


def _emit(tc, io):
    nc = tc.nc
    with ExitStack() as ctx, nc.allow_low_precision(reason="fp32r matmul operand rounding"):
        wp = ctx.enter_context(tc.tile_pool(name="wp", bufs=1))
        sb = ctx.enter_context(tc.tile_pool(name="sb", bufs=1))
        sp = ctx.enter_context(tc.tile_pool(name="sp", bufs=2))
        ps_l = ctx.enter_context(tc.tile_pool(name="ps_l", bufs=2, space="PSUM"))
        ps_m = ctx.enter_context(tc.tile_pool(name="ps_m", bufs=4, space="PSUM"))

        # ---- persistent weights / constants ----
        def load_w(key):
            ts = []
            for kt in range(CT):
                t = wp.tile([128, C], F32R, name=f"{key}{kt}", tag=f"{key}{kt}")
                nc.sync.dma_start(t[:], io[key][kt * 128 : (kt + 1) * 128, :])
                ts.append(t)
            return ts

        # PE warmup: the array sits idle ~13us waiting on x-DMA + GN stats and
        # would start cold (HAM 1.2GHz). Fill the window with junk fp32 matmuls
        # so the 3.4us activity window is warm before real work arrives.
        wsrc = wp.tile([128, 512], F32, name="wsrc", tag="wsrc")
        nc.vector.memset(wsrc[:], 0.0)
        warm_ps = ps_m.tile([128, 512], F32, name="warm_ps", tag="mm")
        for _ in range(4):
            nc.tensor.matmul(
                warm_ps[:], wsrc[:, 0:128], wsrc[:], start=True, stop=True
            )

        gmask_sb = []
        for kt in range(CT):
            t = wp.tile([128, G], F32R, name=f"gmask{kt}", tag=f"gmask{kt}")
            nc.sync.dma_start(t[:], io["gmask"][kt * 128 : (kt + 1) * 128, :])
            gmask_sb.append(t)
        gmaskT_sb = wp.tile([G, C], F32R, name="gmaskT", tag="gmaskT")
        nc.sync.dma_start(gmaskT_sb[:], io["gmaskT"][:])
        onescol = wp.tile([128, 1], F32R, name="onescol", tag="onescol")
        nc.sync.dma_start(onescol[:], io["onescol"][:])

        vecs_sb = wp.tile([128, CT * 4], F32, name="vecs", tag="vecs")
        nc.sync.dma_start(
            vecs_sb[:].rearrange("p (t f) -> p t f", t=CT),
            io["vecs"].rearrange("(t p) f -> p t f", p=128),
        )

        def vcol(ct, f):
            return vecs_sb[:, ct * 4 + f : ct * 4 + f + 1]

        ones1 = wp.tile([1, 128], F32R, name="ones1", tag="ones1")
        nc.sync.dma_start(ones1[:], io["ones1"][:])

        wu_sb = load_w("wuT")
        wvo_sb = load_w("wvoT")
        w2_sb = None
        if io.get("w2col") is not None:
            w2_sb = []
            for kt in range(CT):
                t = wp.tile([128, 1], F32R, name=f"w2c{kt}", tag=f"w2c{kt}")
                nc.sync.dma_start(t[:], io["w2col"][kt * 128 : (kt + 1) * 128, :])
                w2_sb.append(t)

        def stats_phase(img):
            # ---- load x ----
                xt = []
                for ct in range(CT):
                    t = sb.tile([128, HW], F32, name=f"xt{ct}", tag=f"xt{ct}", bufs=2)
                    nc.gpsimd.dma_start(t[:], io["x"][img, ct * 128 : (ct + 1) * 128, :])
                    xt.append(t)

                # ---- groupnorm stats: per-channel sum (DVE) and sumsq (ACT) ----
                stat2 = []
                for ct in range(CT):
                    s2 = sb.tile([128, 2], F32R, name=f"stat2_{ct}", tag=f"stat2_{ct}", bufs=2)
                    nc.vector.reduce_sum(s2[:, 0:1], xt[ct][:], axis=AX.X)
                    scr = sp.tile(
                        [128, HW], F32, name="scr", tag=f"scr{ct % 2}", bufs=1
                    )
                    if ct % 2 == 0 and img == 0:
                        nc.scalar.activation(
                            scr[:], xt[ct][:], AF.Square, accum_out=s2[:, 1:2]
                        )
                    else:
                        nc.vector.scalar_tensor_tensor(
                            scr[:], xt[ct][:], 1.0, xt[ct][:],
                            op0=ALU.mult, op1=ALU.mult,
                            accum_out=s2[:, 1:2],
                        )
                    stat2.append(s2)

                # group sums via membership-mask matmul: [8, 2]
                gstat = ps_m.tile([G, 2], F32, name="gstat", tag="mm")
                for ct in range(CT):
                    nc.tensor.matmul(
                        gstat[:],
                        r(gmask_sb[ct][:]),
                        r(stat2[ct][:]),
                        start=(ct == 0),
                        stop=(ct == CT - 1),
                    )
                gs = sb.tile([G, 2], F32, name="gs", tag="gs", bufs=2)
                nc.vector.tensor_copy(gs[:], gstat[:])

                # per-group mean / rstd, packed as grp2 = [mean, rstd]
                grp2 = sb.tile([G, 2], F32R, name="grp2", tag="grp2", bufs=2)
                tmx = sb.tile([G, 4], F32, name="tmx", tag="tmx", bufs=2)
                nc.vector.tensor_scalar_mul(grp2[:, 0:1], gs[:, 0:1], INVCNT)  # mean
                nc.vector.tensor_scalar_mul(tmx[:, 0:1], gs[:, 1:2], INVCNT)  # E[x^2]
                nc.vector.tensor_mul(tmx[:, 1:2], grp2[:, 0:1], grp2[:, 0:1])  # mean^2
                nc.vector.scalar_tensor_tensor(
                    tmx[:, 2:3], tmx[:, 0:1], EPS, tmx[:, 1:2],
                    op0=ALU.add, op1=ALU.subtract,
                )  # var + eps
                nc.vector.reciprocal(tmx[:, 3:4], tmx[:, 2:3])
                nc.scalar.sqrt(grp2[:, 1:2], tmx[:, 3:4])  # rstd

                # broadcast mean/rstd to channels, fold gamma/beta
                ac, bc = [], []
                for ct in range(CT):
                    bcp = ps_m.tile([128, 2], F32, name="bcp", tag="mm")
                    nc.tensor.matmul(
                        bcp[:],
                        r(gmaskT_sb[:, ct * 128 : (ct + 1) * 128]),
                        r(grp2[:]),
                        start=True,
                        stop=True,
                    )
                    a1 = sb.tile([128, 4], F32, name=f"ab{ct}", tag=f"ab{ct}", bufs=2)
                    # a = rstd * gamma ; b = beta - mean * a
                    nc.vector.tensor_mul(a1[:, 0:1], bcp[:, 1:2], vcol(ct, 1))
                    nc.vector.tensor_mul(a1[:, 2:3], bcp[:, 0:1], a1[:, 0:1])
                    nc.vector.tensor_sub(a1[:, 1:2], vcol(ct, 2), a1[:, 2:3])
                    ac.append(a1[:, 0:1])
                    bc.append(a1[:, 1:2])
                return xt, ac, bc

        per_img = [stats_phase(img) for img in range(BLOC)]

        for img in range(BLOC):
            xt, ac, bc = per_img[img]
            # hn = x * a + b   (DVE two-op tensor_scalar)
            hn = []
            for ct in range(CT):
                t = sb.tile([128, HW], F32R, name=f"hn{ct}", tag=f"hn{ct}")
                if ct % 2 == 0:
                    nc.vector.tensor_scalar(
                        t[:], xt[ct][:], ac[ct], bc[ct], op0=ALU.mult, op1=ALU.add
                    )
                else:
                    nc.scalar.activation(
                        t[:], xt[ct][:], AF.Identity, bias=bc[ct], scale=ac[ct]
                    )
                hn.append(t)

            # ---- u projection: u = (scale * wk^T wq) @ hn, so L = u^T hn ----
            u_sb = []
            for cc in range(CT):
                dst = sb.tile([128, HW], F32R, name=f"u{cc}", tag=f"u{cc}")
                accs = [
                    ps_m.tile([128, 512], F32, name="qp", tag="mm")
                    for _ in range(NCH)
                ]
                for kt in range(CT):
                    for nch in range(NCH):
                        nc.tensor.matmul(
                            accs[nch][:],
                            r(wu_sb[kt][:, cc * 128 : (cc + 1) * 128]),
                            r(hn[kt][:, nch * 512 : (nch + 1) * 512]),
                            start=(kt == 0),
                            stop=(kt == CT - 1),
                        )
                for nch in range(NCH):
                    dslice = dst[:, nch * 512 : (nch + 1) * 512]
                    if (cc + nch) % 2 == 0:
                        nc.vector.tensor_copy(dslice, accs[nch][:])
                    else:
                        nc.scalar.copy(dslice, accs[nch][:])
                u_sb.append(dst)

            # ---- v'T: [m, c'] with v' = (wo @ wv) @ hn (projection pre-merged) ----
            vT = [None] * NB

            def emit_vT(mts):
                for mt in mts:
                    dst = sb.tile([128, C], F32R, name=f"vT{mt}", tag=f"vT{mt}")
                    acc = ps_m.tile([128, 512], F32, name="vp", tag="mm")
                    for kt in range(CT):
                        nc.tensor.matmul(
                            acc[:],
                            r(hn[kt][:, mt * 128 : (mt + 1) * 128]),
                            r(wvo_sb[kt][:]),
                            start=(kt == 0),
                            stop=(kt == CT - 1),
                        )
                    if mt % 2 == 0:
                        nc.vector.tensor_copy(dst[:], acc[:])
                    else:
                        nc.scalar.copy(dst[:], acc[:])
                    vT[mt] = dst


            # optional per-m logit offset for nonzero bq: c_m = (scale wk^T bq) . hn[:, m]
            tv_sb = None
            if w2_sb is not None:
                tv_sb = []
                for mt in range(NB):
                    tvp = ps_m.tile([128, 1], F32, name="tvp", tag="mm")
                    for kt in range(CT):
                        nc.tensor.matmul(
                            tvp[:],
                            r(hn[kt][:, mt * 128 : (mt + 1) * 128]),
                            r(w2_sb[kt][:]),
                            start=(kt == 0),
                            stop=(kt == CT - 1),
                        )
                    t = sb.tile([128, 1], F32, name=f"tv{mt}", tag=f"tv{mt}", bufs=2)
                    nc.vector.tensor_copy(t[:], tvp[:])
                    tv_sb.append(t)

            # ---- attention: L^T = hn^T u per m-tile; exp writes attnT from PSUM ----
            attnT = []
            for mt in range(NB):
                t = sb.tile([128, HW], F32R, name=f"attnT{mt}", tag=f"attnT{mt}")
                attnT.append(t)
            for mt in range(NB):
                lpT = ps_l.tile([128, HW], F32, name="lpT", tag="lpT")
                for kt in range(CT):
                    for nch in range(NCH):
                        nc.tensor.matmul(
                            lpT[:, nch * 512 : (nch + 1) * 512],
                            r(hn[kt][:, mt * 128 : (mt + 1) * 128]),
                            r(u_sb[kt][:, nch * 512 : (nch + 1) * 512]),
                            start=(kt == 0),
                            stop=(kt == CT - 1),
                        )
                if tv_sb is not None:
                    nc.scalar.activation(
                        attnT[mt][:], lpT[:], AF.Exp, bias=tv_sb[mt][:]
                    )
                else:
                    nc.scalar.activation(attnT[mt][:], lpT[:], AF.Exp)

            emit_vT(range(NB))
            # softmax denominators: column sums via ones-vector matmuls, then
            # 1/sum broadcast rows rb[h] via outer product + fast reciprocal
            cs_t = []
            for half in range(2):
                hsl = slice(half * 512, (half + 1) * 512)
                cs = ps_m.tile([1, 512], F32, name="cs", tag="mm")
                for mt in range(NB):
                    nc.tensor.matmul(
                        cs[:],
                        r(onescol[:]),
                        r(attnT[mt][:, hsl]),
                        start=(mt == 0),
                        stop=(mt == NB - 1),
                    )
                cs_t.append(cs)
            rb_sb = []
            for half in range(2):
                rrow_sb = sp.tile(
                    [1, 512], F32R, name="rrow_sb", tag="rrow_sb", bufs=2
                )
                nc.vector.tensor_copy(rrow_sb[:], cs_t[half][:])
                rb_ps = ps_m.tile([128, 512], F32, name="rb_ps", tag="mm")
                nc.tensor.matmul(
                    rb_ps[:], r(ones1[:]), r(rrow_sb[:]), start=True, stop=True
                )
                t = sp.tile([128, 512], F32, name=f"rb{half}", tag=f"rb{half}", bufs=1)
                nc.vector.reciprocal_approx_fast(t[:], rb_ps[:])
                rb_sb.append(t)

            # ---- A @ V': directly the projected attention output; epilogue
            # normalizes, adds bias + residual, and streams out ----
            for cc in range(CT):
                accs = [
                    ps_m.tile([128, 512], F32, name="op", tag="mm")
                    for _ in range(2)
                ]
                for mt in range(NB):
                    for half in range(2):
                        nc.tensor.matmul(
                            accs[half][:],
                            r(vT[mt][:, cc * 128 : (cc + 1) * 128]),
                            r(attnT[mt][:, half * 512 : (half + 1) * 512]),
                            start=(mt == 0),
                            stop=(mt == NB - 1),
                        )
                for half in range(2):
                    hsl = slice(half * 512, (half + 1) * 512)
                    on = sp.tile([128, 512], F32, name="on", tag="on", bufs=3)
                    nc.vector.tensor_mul(on[:], accs[half][:], rb_sb[half][:])
                    res = sp.tile([128, 512], F32, name="res", tag="res", bufs=3)
                    nc.vector.scalar_tensor_tensor(
                        res[:],
                        on[:],
                        vcol(cc, 3),
                        xt[cc][:, hsl],
                        op0=ALU.add,
                        op1=ALU.add,
                    )
                    out_eng = nc.sync if (cc + half) % 2 == 0 else nc.gpsimd
                    out_eng.dma_start(
                        io["out"][img, cc * 128 : (cc + 1) * 128, hsl],
                        res[:],
                    )


_NC = {}


def _build(has_bq=False):
    global _NC
    if _NC.get(has_bq) is None:
        nc = bacc.Bacc("TRN2", target_bir_lowering=False, debug=False)
        io = {}
        io["x"] = nc.dram_tensor("x", [BLOC, C, HW], F32, kind="ExternalInput").ap()
        for key in ("wuT", "wvoT"):
            io[key] = nc.dram_tensor(key, [C, C], F32R, kind="ExternalInput").ap()
        if has_bq:
            io["w2col"] = nc.dram_tensor(
                "w2col", [C, 1], F32R, kind="ExternalInput"
            ).ap()
        io["gmask"] = nc.dram_tensor("gmask", [C, G], F32R, kind="ExternalInput").ap()
        io["gmaskT"] = nc.dram_tensor("gmaskT", [G, C], F32R, kind="ExternalInput").ap()
        io["onescol"] = nc.dram_tensor("onescol", [128, 1], F32R, kind="ExternalInput").ap()
        io["ones1"] = nc.dram_tensor("ones1", [1, 128], F32R, kind="ExternalInput").ap()
        io["vecs"] = nc.dram_tensor("vecs", [C, 4], F32, kind="ExternalInput").ap()
        io["out"] = nc.dram_tensor("out", [BLOC, C, HW], F32, kind="ExternalOutput").ap()
        with tile.TileContext(nc, pool_alloc_mode="queue") as tc:
            _emit(tc, io)
        nc.compile()
        _NC[has_bq] = nc
    return _NC[has_bq]


def _host_prep(x, gn_w, gn_b, wq, bq, wk, bk, wv, bv, wo, bo):
    f = np.float32
    wq64 = np.asarray(wq, np.float64)
    wk64 = np.asarray(wk, np.float64)
    wv64 = np.asarray(wv, np.float64)
    wo64 = np.asarray(wo, np.float64)
    has_bq = bool(np.any(np.asarray(bq) != 0))
    shared = {
        "wuT": np.ascontiguousarray(SCALE * (wq64.T @ wk64), dtype=f),
        "wvoT": np.ascontiguousarray((wo64 @ wv64).T, dtype=f),
        "vecs": np.ascontiguousarray(
            np.stack(
                [
                    np.asarray(bq, dtype=f),
                    np.asarray(gn_w, dtype=f),
                    np.asarray(gn_b, dtype=f),
                    (bo + wo @ bv).astype(f),
                ],
                axis=1,
            )
        ),
        "gmask": np.repeat(np.eye(G, dtype=f), GSZ, axis=0),
        "gmaskT": np.ascontiguousarray(np.repeat(np.eye(G, dtype=f), GSZ, axis=0).T),
        "onescol": np.ones((128, 1), dtype=f),
        "ones1": np.ones((1, 128), dtype=f),
    }
    if has_bq:
        shared["w2col"] = np.ascontiguousarray(
            (SCALE * (wk64.T @ np.asarray(bq, np.float64)))[:, None], dtype=f
        )
    xr = np.ascontiguousarray(np.asarray(x, dtype=f).reshape(B, C, HW))
    in_maps = []
    for core in range(NCORES):
        m = dict(shared)
        m["x"] = np.ascontiguousarray(xr[core * BLOC : (core + 1) * BLOC])
        in_maps.append(m)
    return in_maps


def _run(inputs, trace=False, **kw):
    in_maps = _host_prep(**inputs)
    nc = _build(has_bq="w2col" in in_maps[0])
    res = run_bass_kernel_spmd(
        nc, in_maps, core_ids=list(range(NCORES)), trace=trace, **kw
    )
    outs = [np.asarray(res.results[i]["out"]) for i in range(NCORES)]
    full = np.concatenate(outs, axis=0).reshape(B, C, H, W).astype(np.float32)
    return full, res


def kernel(**inputs):
    full, _ = _run(inputs, trace=False)
    return full
